# revision 21
# baseline (speedup 1.0000x reference)
"""LoRA self-attention TRN2 kernel (8 NeuronCores, SPMD) — v10.

Sharding: core c = (b, hp) with b = c // 4 (batch), hp = c % 4 (head group of
4 heads = 256 channels). Each core computes q/k/v projections (+LoRA) for its
256 output channels from the full x[b], runs attention for its 4 heads, and a
partial output projection over its 256 context channels. Host sums the 4
partials per batch element and adds bo.

Numerics (unchanged from v5, rel err 3.7e-3 vs the 2e-2 gate): q/k
projections and the [k,q]-oriented QK^T scores use bf16 hi/lo splits
(s = kh*qh + kl*qh + kh*ql, fp32-grade); the softmax shift m-hat comes from
a single-bf16 [q,k] score pass (error cancels exactly in softmax). P*V and
the output projection run in bf16.  (fp32r/FP22 scores and fp8 correction
terms were both tried and measured/simulated at 4.3e-2 / 5.8e-2 — too
coarse: the near-one-hot softmax needs |score err| << top-2 gap.)

v7-v10 scheduling/structure changes vs v5 (445us -> 407us):
  - m-hat pass matmuls (K=64) run as row-tiled concurrent pairs on PE array
    row groups (0,0)+(64,0): khl stores [kl;kh] (kh in rows 64:128) and the
    pair reads kh from kha rows 0:64 / khl rows 64:128 with the dup'd qhh
    halves as stationaries (HW-validated 1.96x, bit-exact).
  - the m-hat finalize transpose moved off the PE/PSUM (DVE 32x32 block
    transpose + a 3-level DRAM-bounce gather AP), freeing a PSUM bank that
    deepens the score ping-pong to sps bufs=3 - the PE no longer stalls on
    the 626ns ACT exp per kt (2 matmuls = 432ns < exp).
  - 36 junk warm-up matmuls keep the PE HAM activity monitor hot through
    the initial x DMA (the clock gate otherwise ramps 0.65->2.4GHz on the
    first projection groups); x streams as 512-col chunks for ns0/ns1 and
    one 1024-col DMA per tile after (each trigger costs ~620ns sequencer).
  - normalize chain shortened: Z-row copy -> reciprocal on [1,512] ->
    gpsimd broadcast of the reciprocal -> multiply straight from PSUM
    (tensor ops may read at most ONE PSUM operand; custom-DVE ops like
    reciprocal_approx_fast read PSUM wrong - keep their inputs in SBUF).

Known-negative results (measured): inline PV in the score stream (in-order
PE queue head-blocks on exp), outproj sharing one [128,1024] PSUM tile,
moving copies onto the ACT queue (it paces attention), denser m-hat
interleave, fp32r anywhere in the q/k/score path.
"""
import sys

sys.path.insert(0, "/opt/trn_rl_repo")

from contextlib import ExitStack

import numpy as np
import ml_dtypes

import concourse.bass as bass
import concourse.tile as tile
from concourse import bacc, mybir
from concourse.bass import ts
from concourse.bass_utils import run_bass_kernel_spmd

F32 = mybir.dt.float32
BF16 = mybir.dt.bfloat16
bf16 = ml_dtypes.bfloat16
AX = mybir.AxisListType
Exp = mybir.ActivationFunctionType.Exp
MULT = mybir.AluOpType.mult
SUB = mybir.AluOpType.subtract
MAX = mybir.AluOpType.max

T = 2048          # sequence length
E = 1024          # embed
OL = 256          # local output channels (4 heads)
D = 64            # head dim
NH = 4            # local heads
R = 8             # lora rank
CI = 8            # contraction chunks of 128 over E
NS = 4            # 512-wide slices over T
TC = 16           # 128-wide tiles over T
VW = 65           # v-aug width per head (64 + ones column)

_CACHE = {}


def _build(lora=True):
    key = ("nc", lora)
    if key in _CACHE:
        return _CACHE[key]

    nc = bacc.Bacc("TRN2", target_bir_lowering=False, debug=False)

    # ---- DRAM I/O ----
    xth_d = nc.dram_tensor("xth", [E, T], BF16, kind="ExternalInput")
    xtl_d = nc.dram_tensor("xtl", [E, T], BF16, kind="ExternalInput")
    w_d = {}
    for p in "qkv":
        for s in "hl":
            w_d[p + s] = nc.dram_tensor(f"w{p}{s}", [E, OL], BF16, kind="ExternalInput")
    woT_d = nc.dram_tensor("woT", [OL, E], BF16, kind="ExternalInput")
    if lora:
        ah_d = nc.dram_tensor("ah", [E, 3 * R], BF16, kind="ExternalInput")
        al_d = nc.dram_tensor("al", [E, 3 * R], BF16, kind="ExternalInput")
        b_d = {}
        for p in "qkv":
            for s in "hl":
                b_d[p + s] = nc.dram_tensor(f"b{p}{s}", [R, OL], BF16,
                                            kind="ExternalInput")
    ident_d = nc.dram_tensor("ident", [128, 128], BF16, kind="ExternalInput")
    outp_d = nc.dram_tensor("outp", [T, E], BF16, kind="ExternalOutput")

    with tile.TileContext(nc) as tc, ExitStack() as ctx:
        # ---------------- persistent tiles ----------------
        # Per-head score operand layouts:
        #   khl[h] [128,T]: rows 0:64 = kT_hi(h), rows 64:128 = kT_lo(h)
        #   qhh[h] [128,T]: rows 0:64 = qT_hi(h), rows 64:128 = qT_hi(h) (dup)
        #   kha[h] [65,T]:  rows 0:64 = kT_hi(h), row 64 = ones
        #   qla[h] [65,T]:  rows 0:64 = qT_lo(h), row 64 = -m-hat(h)
        pers = ctx.enter_context(tc.tile_pool(name="pers", bufs=1))
        khl = [pers.tile([128, T], BF16, name=f"khl{h}") for h in range(NH)]
        kha = [pers.tile([65, T], BF16, name=f"kha{h}") for h in range(NH)]
        qhh = [pers.tile([128, T], BF16, name=f"qhh{h}") for h in range(NH)]
        qla = [pers.tile([65, T], BF16, name=f"qla{h}") for h in range(NH)]
        v16 = [pers.tile([128, NH * VW], BF16, name=f"v16_{i}") for i in range(TC)]
        ident = pers.tile([128, 128], BF16, name="ident")
        ctxT_t = [pers.tile([128, T], BF16, name=f"ctxT{c}") for c in range(2)]
        woT_t = [pers.tile([128, E], BF16, name=f"woT{cc}") for cc in range(2)]

        # attention-scoped pools that must coexist with phase-1 pools
        att = ctx.enter_context(tc.tile_pool(name="att", bufs=2))
        ptp = ctx.enter_context(tc.tile_pool(name="ptp", bufs=1))
        ostp = ctx.enter_context(tc.tile_pool(name="ostp", bufs=2))
        drp = ctx.enter_context(tc.tile_pool(name="drp", bufs=2, space="DRAM"))
        msp = None
        if not lora:
            # 4 banks: [128,1024] f32 x 2 bufs (lora path allocates after ph1)
            msp = ctx.enter_context(tc.tile_pool(name="msp", bufs=2, space="PSUM"))

        # preload the exp table so the first real ACTIVATE isn't stalled
        tjunk0 = att.tile([128, 1], BF16, tag="tj0", name="tjunk0", bufs=1)
        nc.scalar.activation(out=tjunk0, in_=ident[:, 0:1], func=Exp, scale=1.0)

        # PE warm-up on never-written junk tiles: keeps the HAM activity
        # monitor hot through the initial x DMA so the first real matmul
        # group runs at full clock instead of ramping from the idle gate
        wjs = att.tile([128, 512], BF16, tag="wjs", name="wjs", bufs=1)
        wjm = att.tile([128, 512], BF16, tag="wjm", name="wjm", bufs=1)
        nc.vector.memset(wjs, 0.0)
        nc.vector.memset(wjm, 0.0)

        # m-hat pass, generator-style so callers interleave it into other
        # PE work at qt granularity.  ~4 matmuls + 2 ttr per step.
        def mhat_head_gen(h):
            rm16a = att.tile([128, 16], F32, tag="rm16a", name=f"rm16a_{h}")
            rm16b = att.tile([128, 16], F32, tag="rm16b", name=f"rm16b_{h}")
            for qt in range(TC):
                for half, rm in ((0, rm16a), (1, rm16b)):
                    # concurrent row-tiled pair: rows 0:64 (kh from kha) and
                    # rows 64:128 (kh from khl) of the PE array run together
                    ms = msp.tile([128, 1024], F32, tag="ms", name="ms")
                    nc.tensor.matmul(ms[:, 0:512],
                                     qhh[h][0:64, ts(qt, 128)],
                                     kha[h][0:64, ts(2 * half, 512)],
                                     start=True, stop=True)
                    nc.tensor.matmul(ms[:, 512:1024],
                                     qhh[h][64:128, ts(qt, 128)],
                                     khl[h][64:128, ts(2 * half + 1, 512)],
                                     start=True, stop=True)
                    nc.vector.reduce_max(out=rm[:, qt:qt + 1], in_=ms, axis=AX.X)
                yield
            # finalize: -m-hat -> bf16 -> DVE 32x32 block transpose -> DRAM
            # bounce with a 3-level gather AP: qla[h][64, qt*128+q] =
            # rm16s[q, qt].  Block r of strT holds strT[32r+qt, qlo] =
            # -mhat[32r+qlo, qt]; the bounce AP (qt, r, qlo) linearizes it.
            rm16 = att.tile([128, 16], F32, tag="rm16", name=f"rm16_{h}")
            nc.vector.tensor_max(rm16, rm16a, rm16b)
            rm32s = att.tile([128, 32], BF16, tag="rm32s", name=f"rm32s_{h}")
            nc.vector.tensor_scalar_mul(rm32s[:, 0:16], rm16, -1.0)
            strT = att.tile([128, 32], BF16, tag="strT", name=f"strT_{h}")
            nc.vector.transpose(strT, rm32s)
            dr = drp.tile([128, 32], BF16, tag="mh_dr", name="mh_dr")
            nc.sync.dma_start(out=dr, in_=strT)
            src = bass.AP(tensor=dr.tensor, offset=dr.offset,
                          ap=[[32, 16], [32 * 32, 4], [1, 32]])
            nc.sync.dma_start(out=qla[h][64:65, :], in_=src)
            yield

        # ---------------- phase 1: projections ----------------
        with ExitStack() as ph1:
            ld = ph1.enter_context(tc.tile_pool(name="ld", bufs=1))
            wpool = ph1.enter_context(tc.tile_pool(name="wpool", bufs=2))
            pps = ph1.enter_context(tc.tile_pool(name="pps", bufs=2, space="PSUM"))

            for _ in range(3):
                wps = pps.tile([128, 512], F32, tag="proj", name="warm")
                for i in range(18):
                    nc.tensor.matmul(wps, wjs[:, 0:128], wjm,
                                     start=(i == 0), stop=(i == 17))
            if lora:
                upsp = ph1.enter_context(tc.tile_pool(name="upsp", bufs=1,
                                                      space="PSUM"))
            vtrp = ph1.enter_context(tc.tile_pool(name="vtrp", bufs=1, space="PSUM"))

            for h in range(NH):
                nc.vector.memset(kha[h][64:65, :], 1.0)

            xth_t, xtl_t, ah_t, al_t = [], [], [], []
            for ci in range(CI):
                xth_t.append(ld.tile([128, T], BF16, name=f"xth{ci}"))
                xtl_t.append(ld.tile([128, T], BF16, name=f"xtl{ci}"))
                if lora:
                    t_ = ld.tile([128, 3 * R], BF16, name=f"ah{ci}")
                    nc.sync.dma_start(out=t_, in_=ah_d[ts(ci, 128), :])
                    ah_t.append(t_)
                    t_ = ld.tile([128, 3 * R], BF16, name=f"al{ci}")
                    nc.sync.dma_start(out=t_, in_=al_d[ts(ci, 128), :])
                    al_t.append(t_)
            w_tiles = {}

            def load_w(p, cis=tuple(range(CI))):
                eng = nc.scalar if p == "v" else nc.sync
                wh_t, wl_t = w_tiles.setdefault(p, ([], []))
                for ci in cis:
                    t_ = wpool.tile([128, OL], BF16, tag=f"wh{ci}", name=f"wh{ci}")
                    eng.dma_start(out=t_, in_=w_d[p + "h"][ts(ci, 128), :])
                    wh_t.append(t_)
                    if p != "v":
                        t_ = wpool.tile([128, OL], BF16, tag=f"wl{ci}", name=f"wl{ci}")
                        eng.dma_start(out=t_, in_=w_d[p + "l"][ts(ci, 128), :])
                        wl_t.append(t_)

            # Wq + the ns=0 x chunks first (split across the SP and ACT DMA
            # queues) so the first accumulation group starts ~10us in; the
            # rest of x as one big chunk per tile to bound sequencer time.
            load_w("q", cis=(0, 1))
            for ns in range(2):
                sl = ts(ns, 512)
                for ci in range(CI):
                    nc.sync.dma_start(out=xth_t[ci][:, sl], in_=xth_d[ts(ci, 128), sl])
                    nc.scalar.dma_start(out=xtl_t[ci][:, sl], in_=xtl_d[ts(ci, 128), sl])
                if ns == 0:
                    load_w("q", cis=tuple(range(2, CI)))
                    load_w("k")
                elif ns == 1:
                    load_w("v")
            # remaining half as one DMA per tile (each trigger costs ~620ns
            # of sequencer time, so fewer/bigger beats chunked back here)
            for ci in range(CI):
                nc.sync.dma_start(out=xth_t[ci][:, 1024:T],
                                  in_=xth_d[ts(ci, 128), 1024:T])
                nc.scalar.dma_start(out=xtl_t[ci][:, 1024:T],
                                    in_=xtl_d[ts(ci, 128), 1024:T])
                if ci == 4:
                    nc.scalar.dma_start(out=ident, in_=ident_d[:, :])
                    for cc in range(2):
                        nc.scalar.dma_start(out=woT_t[cc], in_=woT_d[ts(cc, 128), :])

            u_bf = {}
            b_t = {}
            if lora:
                for key2, d in b_d.items():
                    t_ = ld.tile([R, OL], BF16, name=f"b{key2}")
                    nc.sync.dma_start(out=t_, in_=d[:, :])
                    b_t[key2] = t_

                # u_all = x @ A_all (split3), shared M=24 pass
                ups = upsp.tile([3 * R, T], F32, name="ups")
                for ns in range(NS):
                    sl = ts(ns, 512)
                    n_mm = 3 * CI
                    i = 0
                    for ci in range(CI):
                        for a_t, x_t in ((ah_t[ci], xth_t[ci]), (ah_t[ci], xtl_t[ci]),
                                         (al_t[ci], xth_t[ci])):
                            nc.tensor.matmul(ups[:, sl], a_t, x_t[:, sl],
                                             start=(i == 0), stop=(i == n_mm - 1))
                            i += 1
                uf = ld.tile([3 * R, T], F32, name="uf")
                nc.any.tensor_copy(uf, ups)
                for pi, p in enumerate("qkv"):
                    upf = ld.tile([R, T], F32, tag="upf", name=f"u{p}f")
                    nc.sync.dma_start(out=upf, in_=uf[pi * R:(pi + 1) * R, :])
                    uh = ld.tile([R, T], BF16, name=f"u{p}h")
                    ul = ld.tile([R, T], BF16, name=f"u{p}l")
                    nc.vector.tensor_copy(uh, upf)
                    nc.vector.tensor_sub(ul, upf, uh)
                    u_bf[p + "h"], u_bf[p + "l"] = uh, ul

            # --- projections, transposed layout [OL, T] ---
            # q/k run ns-major-interleaved so the PE tracks the chunked x
            # DMA stream instead of draining it per-projection; v follows
            # with m-hat head 0 interleaved.
            mh0 = None
            vth_t = None

            def emit_group(p, oc, ns):
                wh_t, wl_t = w_tiles[p]
                osl = ts(oc, 128)
                h0, h1 = 2 * oc, 2 * oc + 1
                sl = ts(ns, 512)
                ps = pps.tile([128, 512], F32, tag="proj", name="proj")
                if p == "v":
                    base = [(wh_t[ci], xth_t[ci]) for ci in range(CI)]
                else:
                    base = []
                    for ci in range(CI):
                        base += [(wh_t[ci], xth_t[ci]), (wh_t[ci], xtl_t[ci]),
                                 (wl_t[ci], xth_t[ci])]
                seq = [(a[:, osl], b_[:, sl]) for a, b_ in base]
                if lora:
                    seq += [(b_t[p + "h"][:, osl], u_bf[p + "h"][:, sl]),
                            (b_t[p + "h"][:, osl], u_bf[p + "l"][:, sl]),
                            (b_t[p + "l"][:, osl], u_bf[p + "h"][:, sl])]
                for i, (a, b_) in enumerate(seq):
                    nc.tensor.matmul(ps, a, b_, start=(i == 0),
                                     stop=(i == len(seq) - 1))
                if p == "v":
                    nc.any.tensor_copy(vth_t[oc][:, sl], ps)
                    if mh0 is not None:
                        next(mh0, None)
                        next(mh0, None)
                elif p == "q":
                    for h, rows in ((h0, ps[0:64, :]), (h1, ps[64:128, :])):
                        nc.any.tensor_copy(qhh[h][0:64, sl], rows)
                        nc.any.tensor_copy(qhh[h][64:128, sl], rows)
                        nc.vector.tensor_sub(qla[h][0:64, sl], rows,
                                             qhh[h][0:64, sl])
                else:
                    # khl rows: 0:64 = kl, 64:128 = kh (kh upper so the m-hat
                    # pass can row-pair: rows 0:64 from kha, 64:128 from khl)
                    for h, rows in ((h0, ps[0:64, :]), (h1, ps[64:128, :])):
                        nc.any.tensor_copy(khl[h][64:128, sl], rows)
                        nc.any.tensor_copy(kha[h][0:64, sl], rows)
                        nc.vector.tensor_sub(khl[h][0:64, sl], rows,
                                             khl[h][64:128, sl])

            for ns in range(NS):
                for p in "qk":
                    for oc in range(2):
                        emit_group(p, oc, ns)
            vth_t = [wpool.tile([128, T], BF16, tag=f"vth{c}", name=f"vth{c}",
                                bufs=1) for c in range(2)]
            if not lora:
                mh0 = mhat_head_gen(0)
            for oc in range(2):
                for ns in range(NS):
                    emit_group("v", oc, ns)
            if True:
                    # v16 tiles: per head 64 v-cols + a ones column (Z trick)
                    for tci in range(TC):
                        nc.vector.memset(v16[tci], 1.0)
                    for oc in range(2):
                        for tci in range(TC):
                            tp = vtrp.tile([128, 128], BF16, tag="vtr", name="vtr")
                            nc.tensor.transpose(tp, vth_t[oc][:, ts(tci, 128)], ident)
                            h0, h1 = 2 * oc, 2 * oc + 1
                            nc.any.tensor_copy(v16[tci][:, h0 * VW:h0 * VW + 64],
                                               tp[:, 0:64])
                            nc.any.tensor_copy(v16[tci][:, h1 * VW:h1 * VW + 64],
                                               tp[:, 64:128])
                            if mh0 is not None and tci % 2 == 0:
                                next(mh0, None)

        # ---------------- phase 3+4: attention + output projection ----------------
        if lora:
            msp = ctx.enter_context(tc.tile_pool(name="msp", bufs=2, space="PSUM"))
            mh0 = mhat_head_gen(0)
        # drain whatever is left of m-hat head 0 (its finalize uses mtp,
        # which only exists after the phase-1 PSUM pools are freed)
        if mh0 is not None:
            for _ in mh0:
                pass

        with ExitStack() as ph3:
            sps = ph3.enter_context(tc.tile_pool(name="sps", bufs=3, space="PSUM"))
            cps = ph3.enter_context(tc.tile_pool(name="cps", bufs=1, space="PSUM"))

            def outproj_qb(qb):
                for tci in range(4 * qb, 4 * qb + 4):
                    tsl = ts(tci, 128)
                    ost = ostp.tile([128, E], BF16, tag="ost", name="ost")
                    for no in range(2):
                        # msp is idle while head 3 runs (no next-head m-hat);
                        # borrow its banks so scores never wait behind outproj
                        op_w = msp.tile([128, 1024], F32, tag="ms", name="op")
                        op_t = op_w[:, 0:512]
                        for cc in range(2):
                            nc.tensor.matmul(op_t, ctxT_t[cc][:, tsl],
                                             woT_t[cc][:, ts(no, 512)],
                                             start=(cc == 0), stop=(cc == 1))
                        nc.vector.tensor_copy(ost[:, ts(no, 512)], op_t)
                        nc.sync.dma_start(out=outp_d[tsl, ts(no, 512)],
                                          in_=ost[:, ts(no, 512)])

            for h in range(NH):
                ch = h // 2
                pr = (h % 2) * 64
                mh = mhat_head_gen(h + 1) if h + 1 < NH else None

                for qb in range(NS):
                    qsl = ts(qb, 512)
                    # --- scores: K-stacked with fused -m-hat -> exp ---
                    pT = [ptp.tile([128, 512], BF16, tag=f"pt{i}", name=f"pt{i}")
                          for i in range(TC)]
                    for kt in range(TC):
                        st = sps.tile([128, 512], F32, tag="st", name="st")
                        # kh·qh + kl·qh in one K=128 matmul (qh duplicated)
                        nc.tensor.matmul(st, khl[h][:, ts(kt, 128)], qhh[h][:, qsl],
                                         start=True, stop=False)
                        # kh·ql + ones·(-m-hat), K=65
                        nc.tensor.matmul(st, kha[h][:, ts(kt, 128)],
                                         qla[h][:, qsl], start=False, stop=True)
                        nc.scalar.activation(out=pT[kt], in_=st, func=Exp, scale=0.125)
                        if mh is not None and qb < NS - 1 and kt % 5 == 1:
                            next(mh, None)
                        if h == NH - 1 and qb > 0 and kt == TC - 1:
                            outproj_qb(qb - 1)
                    # --- PV with ones column ---
                    cxa = cps.tile([VW, 512], F32, tag="cxa", name="cxa")
                    for kt in range(TC):
                        nc.tensor.matmul(cxa, v16[kt][:, h * VW:(h + 1) * VW], pT[kt],
                                         start=(kt == 0), stop=(kt == TC - 1))
                        if mh is not None and qb < NS - 1 and kt % 5 == 3:
                            next(mh, None)
                    # --- normalize by Z (row 64): reciprocal on the Z row,
                    # broadcast the reciprocal, multiply straight from PSUM ---
                    zrow = att.tile([1, 512], F32, tag="zrow", name="zrow")
                    nc.vector.tensor_copy(zrow, cxa[64:65, :])
                    zrcp = att.tile([1, 512], F32, tag="zrcp", name="zrcp")
                    nc.vector.reciprocal_approx_fast(out=zrcp, in_=zrow)
                    rcp_bc = att.tile([64, 512], F32, tag="rcpbc", name="rcp_bc", bufs=1)
                    nc.gpsimd.partition_broadcast(rcp_bc, zrcp, channels=64)
                    nc.vector.tensor_mul(ctxT_t[ch][pr:pr + 64, qsl], cxa[0:64, :],
                                         rcp_bc)
                # drain this head's pipelined m-hat
                if mh is not None:
                    for _ in mh:
                        pass
            outproj_qb(NS - 1)

    nc.compile()
    _CACHE[key] = nc
    return nc


def _split(a):
    h = a.astype(bf16)
    l = (a - h.astype(np.float32)).astype(bf16)
    return h, l


def _shard(inputs, lora):
    x = np.asarray(inputs["x"], np.float32)
    Wo = np.asarray(inputs["Wo"], np.float32)
    ident = np.eye(128, dtype=np.float32).astype(bf16)
    if lora:
        A_all = np.concatenate([np.asarray(inputs["Aq"], np.float32),
                                np.asarray(inputs["Ak"], np.float32),
                                np.asarray(inputs["Av"], np.float32)], axis=1)
        ah, al = _split(A_all)
    in_maps = []
    for core in range(8):
        b, hp = core // 4, core % 4
        o0 = hp * OL
        xT = np.ascontiguousarray(x[b].T)
        xh, xl = _split(xT)
        m = {"xth": xh, "xtl": xl, "ident": ident}
        for p in "qkv":
            W = np.asarray(inputs["W" + p], np.float32)
            Ws = np.ascontiguousarray(W[o0:o0 + OL, :].T)
            m["w%sh" % p], m["w%sl" % p] = _split(Ws)
            if lora:
                B = np.asarray(inputs["B" + p], np.float32)[:, o0:o0 + OL] * 2.0
                m["b%sh" % p], m["b%sl" % p] = _split(B)
        m["woT"] = np.ascontiguousarray(Wo[:, o0:o0 + OL].T).astype(bf16)
        if lora:
            m["ah"], m["al"] = ah, al
        in_maps.append(m)
    return in_maps


def _run(inputs, trace=False, **kw):
    lora = not all(
        np.count_nonzero(np.asarray(inputs["B" + p])) == 0 for p in "qkv")
    nc = _build(lora)
    in_maps = _shard(inputs, lora)
    res = run_bass_kernel_spmd(nc, in_maps, core_ids=list(range(8)), trace=trace, **kw)
    bo = np.asarray(inputs["bo"], np.float32)
    parts = [res.results[c]["outp"].astype(np.float64) for c in range(8)]
    out = np.stack([sum(parts[0:4]), sum(parts[4:8])]) + bo.astype(np.float64)
    return out.astype(np.float32), res


def kernel(**inputs):
    out, _ = _run(inputs)
    return out



# revision 22
# speedup vs baseline: 1.0063x; 1.0063x over previous
"""LoRA self-attention TRN2 kernel (8 NeuronCores, SPMD) — v10.

Sharding: core c = (b, hp) with b = c // 4 (batch), hp = c % 4 (head group of
4 heads = 256 channels). Each core computes q/k/v projections (+LoRA) for its
256 output channels from the full x[b], runs attention for its 4 heads, and a
partial output projection over its 256 context channels. Host sums the 4
partials per batch element and adds bo.

Numerics (unchanged from v5, rel err 3.7e-3 vs the 2e-2 gate): q/k
projections and the [k,q]-oriented QK^T scores use bf16 hi/lo splits
(s = kh*qh + kl*qh + kh*ql, fp32-grade); the softmax shift m-hat comes from
a single-bf16 [q,k] score pass (error cancels exactly in softmax). P*V and
the output projection run in bf16.  (fp32r/FP22 scores and fp8 correction
terms were both tried and measured/simulated at 4.3e-2 / 5.8e-2 — too
coarse: the near-one-hot softmax needs |score err| << top-2 gap.)

v7-v10 scheduling/structure changes vs v5 (445us -> 407us):
  - m-hat pass matmuls (K=64) run as row-tiled concurrent pairs on PE array
    row groups (0,0)+(64,0): khl stores [kl;kh] (kh in rows 64:128) and the
    pair reads kh from kha rows 0:64 / khl rows 64:128 with the dup'd qhh
    halves as stationaries (HW-validated 1.96x, bit-exact).
  - the m-hat finalize transpose moved off the PE/PSUM (DVE 32x32 block
    transpose + a 3-level DRAM-bounce gather AP), freeing a PSUM bank that
    deepens the score ping-pong to sps bufs=3 - the PE no longer stalls on
    the 626ns ACT exp per kt (2 matmuls = 432ns < exp).
  - 36 junk warm-up matmuls keep the PE HAM activity monitor hot through
    the initial x DMA (the clock gate otherwise ramps 0.65->2.4GHz on the
    first projection groups); x streams as 512-col chunks for ns0/ns1 and
    one 1024-col DMA per tile after (each trigger costs ~620ns sequencer).
  - normalize chain shortened: Z-row copy -> reciprocal on [1,512] ->
    gpsimd broadcast of the reciprocal -> multiply straight from PSUM
    (tensor ops may read at most ONE PSUM operand; custom-DVE ops like
    reciprocal_approx_fast read PSUM wrong - keep their inputs in SBUF).

Known-negative results (measured): inline PV in the score stream (in-order
PE queue head-blocks on exp), outproj sharing one [128,1024] PSUM tile,
moving copies onto the ACT queue (it paces attention), denser m-hat
interleave, fp32r anywhere in the q/k/score path.
"""
import sys

sys.path.insert(0, "/opt/trn_rl_repo")

from contextlib import ExitStack

import numpy as np
import ml_dtypes

import concourse.bass as bass
import concourse.tile as tile
from concourse import bacc, mybir
from concourse.bass import ts
from concourse.bass_utils import run_bass_kernel_spmd

F32 = mybir.dt.float32
BF16 = mybir.dt.bfloat16
bf16 = ml_dtypes.bfloat16
AX = mybir.AxisListType
Exp = mybir.ActivationFunctionType.Exp
MULT = mybir.AluOpType.mult
SUB = mybir.AluOpType.subtract
MAX = mybir.AluOpType.max

T = 2048          # sequence length
E = 1024          # embed
OL = 256          # local output channels (4 heads)
D = 64            # head dim
NH = 4            # local heads
R = 8             # lora rank
CI = 8            # contraction chunks of 128 over E
NS = 4            # 512-wide slices over T
TC = 16           # 128-wide tiles over T
VW = 65           # v-aug width per head (64 + ones column)

_CACHE = {}


def _build(lora=True):
    key = ("nc", lora)
    if key in _CACHE:
        return _CACHE[key]

    nc = bacc.Bacc("TRN2", target_bir_lowering=False, debug=False)

    # ---- DRAM I/O ----
    xth_d = nc.dram_tensor("xth", [E, T], BF16, kind="ExternalInput")
    xtl_d = nc.dram_tensor("xtl", [E, T], BF16, kind="ExternalInput")
    w_d = {}
    for p in "qkv":
        for s in "hl":
            w_d[p + s] = nc.dram_tensor(f"w{p}{s}", [E, OL], BF16, kind="ExternalInput")
    woT_d = nc.dram_tensor("woT", [OL, E], BF16, kind="ExternalInput")
    if lora:
        ah_d = nc.dram_tensor("ah", [E, 3 * R], BF16, kind="ExternalInput")
        al_d = nc.dram_tensor("al", [E, 3 * R], BF16, kind="ExternalInput")
        b_d = {}
        for p in "qkv":
            for s in "hl":
                b_d[p + s] = nc.dram_tensor(f"b{p}{s}", [R, OL], BF16,
                                            kind="ExternalInput")
    ident_d = nc.dram_tensor("ident", [128, 128], BF16, kind="ExternalInput")
    outp_d = nc.dram_tensor("outp", [T, E], BF16, kind="ExternalOutput")

    with tile.TileContext(nc) as tc, ExitStack() as ctx:
        # ---------------- persistent tiles ----------------
        # Per-head score operand layouts:
        #   khl[h] [128,T]: rows 0:64 = kT_hi(h), rows 64:128 = kT_lo(h)
        #   qhh[h] [128,T]: rows 0:64 = qT_hi(h), rows 64:128 = qT_hi(h) (dup)
        #   kha[h] [65,T]:  rows 0:64 = kT_hi(h), row 64 = ones
        #   qla[h] [65,T]:  rows 0:64 = qT_lo(h), row 64 = -m-hat(h)
        pers = ctx.enter_context(tc.tile_pool(name="pers", bufs=1))
        khl = [pers.tile([128, T], BF16, name=f"khl{h}") for h in range(NH)]
        kha = [pers.tile([65, T], BF16, name=f"kha{h}") for h in range(NH)]
        qhh = [pers.tile([128, T], BF16, name=f"qhh{h}") for h in range(NH)]
        qla = [pers.tile([65, T], BF16, name=f"qla{h}") for h in range(NH)]
        v16 = [pers.tile([128, NH * VW], BF16, name=f"v16_{i}") for i in range(TC)]
        ident = pers.tile([128, 128], BF16, name="ident")
        ctxT_t = [pers.tile([128, T], BF16, name=f"ctxT{c}") for c in range(2)]
        woT_t = [pers.tile([128, E], BF16, name=f"woT{cc}") for cc in range(2)]

        # attention-scoped pools that must coexist with phase-1 pools
        att = ctx.enter_context(tc.tile_pool(name="att", bufs=2))
        ptp = ctx.enter_context(tc.tile_pool(name="ptp", bufs=1))
        ostp = ctx.enter_context(tc.tile_pool(name="ostp", bufs=2))
        drp = ctx.enter_context(tc.tile_pool(name="drp", bufs=2, space="DRAM"))
        msp = None
        if not lora:
            # 4 banks: [128,1024] f32 x 2 bufs (lora path allocates after ph1)
            msp = ctx.enter_context(tc.tile_pool(name="msp", bufs=2, space="PSUM"))

        # preload the exp table so the first real ACTIVATE isn't stalled
        tjunk0 = att.tile([128, 1], BF16, tag="tj0", name="tjunk0", bufs=1)
        nc.scalar.activation(out=tjunk0, in_=ident[:, 0:1], func=Exp, scale=1.0)

        # PE warm-up on never-written junk tiles: keeps the HAM activity
        # monitor hot through the initial x DMA so the first real matmul
        # group runs at full clock instead of ramping from the idle gate
        wjs = att.tile([128, 512], BF16, tag="wjs", name="wjs", bufs=1)
        wjm = att.tile([128, 512], BF16, tag="wjm", name="wjm", bufs=1)
        nc.vector.memset(wjs, 0.0)
        nc.vector.memset(wjm, 0.0)

        # m-hat pass, generator-style so callers interleave it into other
        # PE work at qt granularity.  ~4 matmuls + 2 ttr per step.
        def mhat_head_gen(h):
            rm16a = att.tile([128, 16], F32, tag="rm16a", name=f"rm16a_{h}")
            rm16b = att.tile([128, 16], F32, tag="rm16b", name=f"rm16b_{h}")
            for qt in range(TC):
                for half, rm in ((0, rm16a), (1, rm16b)):
                    # concurrent row-tiled pair: rows 0:64 (kh from kha) and
                    # rows 64:128 (kh from khl) of the PE array run together
                    ms = msp.tile([128, 1024], F32, tag="ms", name="ms")
                    nc.tensor.matmul(ms[:, 0:512],
                                     qhh[h][0:64, ts(qt, 128)],
                                     kha[h][0:64, ts(2 * half, 512)],
                                     start=True, stop=True)
                    nc.tensor.matmul(ms[:, 512:1024],
                                     qhh[h][64:128, ts(qt, 128)],
                                     khl[h][64:128, ts(2 * half + 1, 512)],
                                     start=True, stop=True)
                    nc.vector.reduce_max(out=rm[:, qt:qt + 1], in_=ms, axis=AX.X)
                yield
            # finalize: -m-hat -> bf16 -> DVE 32x32 block transpose -> DRAM
            # bounce with a 3-level gather AP: qla[h][64, qt*128+q] =
            # rm16s[q, qt].  Block r of strT holds strT[32r+qt, qlo] =
            # -mhat[32r+qlo, qt]; the bounce AP (qt, r, qlo) linearizes it.
            rm16 = att.tile([128, 16], F32, tag="rm16", name=f"rm16_{h}")
            nc.vector.tensor_max(rm16, rm16a, rm16b)
            rm32s = att.tile([128, 32], BF16, tag="rm32s", name=f"rm32s_{h}")
            nc.vector.tensor_scalar_mul(rm32s[:, 0:16], rm16, -1.0)
            strT = att.tile([128, 32], BF16, tag="strT", name=f"strT_{h}")
            nc.vector.transpose(strT, rm32s)
            dr = drp.tile([128, 32], BF16, tag="mh_dr", name="mh_dr")
            nc.sync.dma_start(out=dr, in_=strT)
            src = bass.AP(tensor=dr.tensor, offset=dr.offset,
                          ap=[[32, 16], [32 * 32, 4], [1, 32]])
            nc.sync.dma_start(out=qla[h][64:65, :], in_=src)
            yield

        # ---------------- phase 1: projections ----------------
        with ExitStack() as ph1:
            ld = ph1.enter_context(tc.tile_pool(name="ld", bufs=1))
            wpool = ph1.enter_context(tc.tile_pool(name="wpool", bufs=2))
            pps = ph1.enter_context(tc.tile_pool(name="pps", bufs=2, space="PSUM"))

            for _ in range(2):
                wps = pps.tile([128, 512], F32, tag="proj", name="warm")
                for i in range(18):
                    nc.tensor.matmul(wps, wjs[:, 0:128], wjm,
                                     start=(i == 0), stop=(i == 17))
            if lora:
                upsp = ph1.enter_context(tc.tile_pool(name="upsp", bufs=1,
                                                      space="PSUM"))
            vtrp = ph1.enter_context(tc.tile_pool(name="vtrp", bufs=1, space="PSUM"))

            for h in range(NH):
                nc.vector.memset(kha[h][64:65, :], 1.0)

            xth_t, xtl_t, ah_t, al_t = [], [], [], []
            for ci in range(CI):
                xth_t.append(ld.tile([128, T], BF16, name=f"xth{ci}"))
                xtl_t.append(ld.tile([128, T], BF16, name=f"xtl{ci}"))
                if lora:
                    t_ = ld.tile([128, 3 * R], BF16, name=f"ah{ci}")
                    nc.sync.dma_start(out=t_, in_=ah_d[ts(ci, 128), :])
                    ah_t.append(t_)
                    t_ = ld.tile([128, 3 * R], BF16, name=f"al{ci}")
                    nc.sync.dma_start(out=t_, in_=al_d[ts(ci, 128), :])
                    al_t.append(t_)
            w_tiles = {}

            def load_w(p, cis=tuple(range(CI))):
                eng = nc.scalar if p == "v" else nc.sync
                wh_t, wl_t = w_tiles.setdefault(p, ([], []))
                for ci in cis:
                    t_ = wpool.tile([128, OL], BF16, tag=f"wh{ci}", name=f"wh{ci}")
                    eng.dma_start(out=t_, in_=w_d[p + "h"][ts(ci, 128), :])
                    wh_t.append(t_)
                    if p != "v":
                        t_ = wpool.tile([128, OL], BF16, tag=f"wl{ci}", name=f"wl{ci}")
                        eng.dma_start(out=t_, in_=w_d[p + "l"][ts(ci, 128), :])
                        wl_t.append(t_)

            # Wq + the ns=0 x chunks first (split across the SP and ACT DMA
            # queues) so the first accumulation group starts ~10us in; the
            # rest of x as one big chunk per tile to bound sequencer time.
            load_w("q", cis=(0, 1))
            for ns in range(2):
                sl = ts(ns, 512)
                for ci in range(CI):
                    nc.sync.dma_start(out=xth_t[ci][:, sl], in_=xth_d[ts(ci, 128), sl])
                    nc.scalar.dma_start(out=xtl_t[ci][:, sl], in_=xtl_d[ts(ci, 128), sl])
                if ns == 0:
                    load_w("q", cis=tuple(range(2, CI)))
                    load_w("k")
                elif ns == 1:
                    load_w("v")
            # remaining half as one DMA per tile (each trigger costs ~620ns
            # of sequencer time, so fewer/bigger beats chunked back here)
            for ci in range(CI):
                nc.sync.dma_start(out=xth_t[ci][:, 1024:T],
                                  in_=xth_d[ts(ci, 128), 1024:T])
                nc.scalar.dma_start(out=xtl_t[ci][:, 1024:T],
                                    in_=xtl_d[ts(ci, 128), 1024:T])
                if ci == 4:
                    nc.scalar.dma_start(out=ident, in_=ident_d[:, :])
                    for cc in range(2):
                        nc.scalar.dma_start(out=woT_t[cc], in_=woT_d[ts(cc, 128), :])

            u_bf = {}
            b_t = {}
            if lora:
                for key2, d in b_d.items():
                    t_ = ld.tile([R, OL], BF16, name=f"b{key2}")
                    nc.sync.dma_start(out=t_, in_=d[:, :])
                    b_t[key2] = t_

                # u_all = x @ A_all (split3), shared M=24 pass
                ups = upsp.tile([3 * R, T], F32, name="ups")
                for ns in range(NS):
                    sl = ts(ns, 512)
                    n_mm = 3 * CI
                    i = 0
                    for ci in range(CI):
                        for a_t, x_t in ((ah_t[ci], xth_t[ci]), (ah_t[ci], xtl_t[ci]),
                                         (al_t[ci], xth_t[ci])):
                            nc.tensor.matmul(ups[:, sl], a_t, x_t[:, sl],
                                             start=(i == 0), stop=(i == n_mm - 1))
                            i += 1
                uf = ld.tile([3 * R, T], F32, name="uf")
                nc.any.tensor_copy(uf, ups)
                for pi, p in enumerate("qkv"):
                    upf = ld.tile([R, T], F32, tag="upf", name=f"u{p}f")
                    nc.sync.dma_start(out=upf, in_=uf[pi * R:(pi + 1) * R, :])
                    uh = ld.tile([R, T], BF16, name=f"u{p}h")
                    ul = ld.tile([R, T], BF16, name=f"u{p}l")
                    nc.vector.tensor_copy(uh, upf)
                    nc.vector.tensor_sub(ul, upf, uh)
                    u_bf[p + "h"], u_bf[p + "l"] = uh, ul

            # --- projections, transposed layout [OL, T] ---
            # q/k run ns-major-interleaved so the PE tracks the chunked x
            # DMA stream instead of draining it per-projection; v follows
            # with m-hat head 0 interleaved.
            mh0 = None
            vth_t = None

            def emit_group(p, oc, ns):
                wh_t, wl_t = w_tiles[p]
                osl = ts(oc, 128)
                h0, h1 = 2 * oc, 2 * oc + 1
                sl = ts(ns, 512)
                ps = pps.tile([128, 512], F32, tag="proj", name="proj")
                if p == "v":
                    base = [(wh_t[ci], xth_t[ci]) for ci in range(CI)]
                else:
                    base = []
                    for ci in range(CI):
                        base += [(wh_t[ci], xth_t[ci]), (wh_t[ci], xtl_t[ci]),
                                 (wl_t[ci], xth_t[ci])]
                seq = [(a[:, osl], b_[:, sl]) for a, b_ in base]
                if lora:
                    seq += [(b_t[p + "h"][:, osl], u_bf[p + "h"][:, sl]),
                            (b_t[p + "h"][:, osl], u_bf[p + "l"][:, sl]),
                            (b_t[p + "l"][:, osl], u_bf[p + "h"][:, sl])]
                for i, (a, b_) in enumerate(seq):
                    nc.tensor.matmul(ps, a, b_, start=(i == 0),
                                     stop=(i == len(seq) - 1))
                if p == "v":
                    nc.any.tensor_copy(vth_t[oc][:, sl], ps)
                    if mh0 is not None:
                        next(mh0, None)
                        next(mh0, None)
                elif p == "q":
                    for h, rows in ((h0, ps[0:64, :]), (h1, ps[64:128, :])):
                        nc.any.tensor_copy(qhh[h][0:64, sl], rows)
                        nc.any.tensor_copy(qhh[h][64:128, sl], rows)
                        nc.vector.tensor_sub(qla[h][0:64, sl], rows,
                                             qhh[h][0:64, sl])
                else:
                    # khl rows: 0:64 = kl, 64:128 = kh (kh upper so the m-hat
                    # pass can row-pair: rows 0:64 from kha, 64:128 from khl)
                    for h, rows in ((h0, ps[0:64, :]), (h1, ps[64:128, :])):
                        nc.any.tensor_copy(khl[h][64:128, sl], rows)
                        nc.any.tensor_copy(kha[h][0:64, sl], rows)
                        nc.vector.tensor_sub(khl[h][0:64, sl], rows,
                                             khl[h][64:128, sl])

            for ns in range(NS):
                for p in "qk":
                    for oc in range(2):
                        emit_group(p, oc, ns)
            vth_t = [wpool.tile([128, T], BF16, tag=f"vth{c}", name=f"vth{c}",
                                bufs=1) for c in range(2)]
            if not lora:
                mh0 = mhat_head_gen(0)
            for oc in range(2):
                for ns in range(NS):
                    emit_group("v", oc, ns)
            if True:
                    # v16 tiles: per head 64 v-cols + a ones column (Z trick)
                    for tci in range(TC):
                        nc.vector.memset(v16[tci], 1.0)
                    for oc in range(2):
                        for tci in range(TC):
                            tp = vtrp.tile([128, 128], BF16, tag="vtr", name="vtr")
                            nc.tensor.transpose(tp, vth_t[oc][:, ts(tci, 128)], ident)
                            h0, h1 = 2 * oc, 2 * oc + 1
                            nc.any.tensor_copy(v16[tci][:, h0 * VW:h0 * VW + 64],
                                               tp[:, 0:64])
                            nc.any.tensor_copy(v16[tci][:, h1 * VW:h1 * VW + 64],
                                               tp[:, 64:128])
                            if mh0 is not None and tci % 2 == 0:
                                next(mh0, None)

        # ---------------- phase 3+4: attention + output projection ----------------
        if lora:
            msp = ctx.enter_context(tc.tile_pool(name="msp", bufs=2, space="PSUM"))
            mh0 = mhat_head_gen(0)
        # drain whatever is left of m-hat head 0 (its finalize uses mtp,
        # which only exists after the phase-1 PSUM pools are freed)
        if mh0 is not None:
            for _ in mh0:
                pass

        with ExitStack() as ph3:
            sps = ph3.enter_context(tc.tile_pool(name="sps", bufs=3, space="PSUM"))
            cps = ph3.enter_context(tc.tile_pool(name="cps", bufs=1, space="PSUM"))

            def outproj_qb(qb):
                for tci in range(4 * qb, 4 * qb + 4):
                    tsl = ts(tci, 128)
                    ost = ostp.tile([128, E], BF16, tag="ost", name="ost")
                    for no in range(2):
                        # msp is idle while head 3 runs (no next-head m-hat);
                        # borrow its banks so scores never wait behind outproj
                        op_w = msp.tile([128, 1024], F32, tag="ms", name="op")
                        op_t = op_w[:, 0:512]
                        for cc in range(2):
                            nc.tensor.matmul(op_t, ctxT_t[cc][:, tsl],
                                             woT_t[cc][:, ts(no, 512)],
                                             start=(cc == 0), stop=(cc == 1))
                        nc.vector.tensor_copy(ost[:, ts(no, 512)], op_t)
                    nc.sync.dma_start(out=outp_d[tsl, :], in_=ost)

            for h in range(NH):
                ch = h // 2
                pr = (h % 2) * 64
                mh = mhat_head_gen(h + 1) if h + 1 < NH else None

                for qb in range(NS):
                    qsl = ts(qb, 512)
                    # --- scores: K-stacked with fused -m-hat -> exp ---
                    pT = [ptp.tile([128, 512], BF16, tag=f"pt{i}", name=f"pt{i}")
                          for i in range(TC)]
                    for kt in range(TC):
                        st = sps.tile([128, 512], F32, tag="st", name="st")
                        # kh·qh + kl·qh in one K=128 matmul (qh duplicated)
                        nc.tensor.matmul(st, khl[h][:, ts(kt, 128)], qhh[h][:, qsl],
                                         start=True, stop=False)
                        # kh·ql + ones·(-m-hat), K=65
                        nc.tensor.matmul(st, kha[h][:, ts(kt, 128)],
                                         qla[h][:, qsl], start=False, stop=True)
                        nc.scalar.activation(out=pT[kt], in_=st, func=Exp, scale=0.125)
                        if mh is not None and qb < NS - 1 and kt % 5 == 1:
                            next(mh, None)
                        if h == NH - 1 and qb > 0 and kt == TC - 1:
                            outproj_qb(qb - 1)
                    # --- PV with ones column ---
                    cxa = cps.tile([VW, 512], F32, tag="cxa", name="cxa")
                    for kt in range(TC):
                        nc.tensor.matmul(cxa, v16[kt][:, h * VW:(h + 1) * VW], pT[kt],
                                         start=(kt == 0), stop=(kt == TC - 1))
                        if mh is not None and qb < NS - 1 and kt % 5 == 3:
                            next(mh, None)
                    # --- normalize by Z (row 64): reciprocal on the Z row,
                    # broadcast the reciprocal, multiply straight from PSUM ---
                    zrow = att.tile([1, 512], F32, tag="zrow", name="zrow")
                    nc.vector.tensor_copy(zrow, cxa[64:65, :])
                    zrcp = att.tile([1, 512], F32, tag="zrcp", name="zrcp")
                    nc.vector.reciprocal_approx_fast(out=zrcp, in_=zrow)
                    rcp_bc = att.tile([64, 512], F32, tag="rcpbc", name="rcp_bc", bufs=1)
                    nc.gpsimd.partition_broadcast(rcp_bc, zrcp, channels=64)
                    nc.vector.tensor_mul(ctxT_t[ch][pr:pr + 64, qsl], cxa[0:64, :],
                                         rcp_bc)
                # drain this head's pipelined m-hat
                if mh is not None:
                    for _ in mh:
                        pass
            outproj_qb(NS - 1)

    nc.compile()
    _CACHE[key] = nc
    return nc


def _split(a):
    h = a.astype(bf16)
    l = (a - h.astype(np.float32)).astype(bf16)
    return h, l


def _shard(inputs, lora):
    x = np.asarray(inputs["x"], np.float32)
    Wo = np.asarray(inputs["Wo"], np.float32)
    ident = np.eye(128, dtype=np.float32).astype(bf16)
    if lora:
        A_all = np.concatenate([np.asarray(inputs["Aq"], np.float32),
                                np.asarray(inputs["Ak"], np.float32),
                                np.asarray(inputs["Av"], np.float32)], axis=1)
        ah, al = _split(A_all)
    in_maps = []
    for core in range(8):
        b, hp = core // 4, core % 4
        o0 = hp * OL
        xT = np.ascontiguousarray(x[b].T)
        xh, xl = _split(xT)
        m = {"xth": xh, "xtl": xl, "ident": ident}
        for p in "qkv":
            W = np.asarray(inputs["W" + p], np.float32)
            Ws = np.ascontiguousarray(W[o0:o0 + OL, :].T)
            m["w%sh" % p], m["w%sl" % p] = _split(Ws)
            if lora:
                B = np.asarray(inputs["B" + p], np.float32)[:, o0:o0 + OL] * 2.0
                m["b%sh" % p], m["b%sl" % p] = _split(B)
        m["woT"] = np.ascontiguousarray(Wo[:, o0:o0 + OL].T).astype(bf16)
        if lora:
            m["ah"], m["al"] = ah, al
        in_maps.append(m)
    return in_maps


def _run(inputs, trace=False, **kw):
    lora = not all(
        np.count_nonzero(np.asarray(inputs["B" + p])) == 0 for p in "qkv")
    nc = _build(lora)
    in_maps = _shard(inputs, lora)
    res = run_bass_kernel_spmd(nc, in_maps, core_ids=list(range(8)), trace=trace, **kw)
    bo = np.asarray(inputs["bo"], np.float32)
    parts = [res.results[c]["outp"].astype(np.float64) for c in range(8)]
    out = np.stack([sum(parts[0:4]), sum(parts[4:8])]) + bo.astype(np.float64)
    return out.astype(np.float32), res


def kernel(**inputs):
    out, _ = _run(inputs)
    return out

